# revision 7
# baseline (speedup 1.0000x reference)
"""PosEncoding TRN2 kernel: out = [c, sin(2^i pi c_j), cos(2^i pi c_j)] interleaved.

Full input coords [1, 2097152, 3] f32 -> output [1, 2097152, 63] f32.
8-way data parallel over the points axis (262144 points/core).

Math per element (freq i in 0..9, feature j in 0..2):
  f_i = frac(c * 2^(i-1))            # exact fp32 chain, see below
  sin(2^i pi c) = Sin(-2pi f_i + pi) # arg in (-pi, pi], ACT valid range
  g_i = frac(f_i + 0.25)
  cos(2^i pi c) = Sin(-2pi g_i + pi)
This walrus build's ISA check rejects AluOpType.mod on every engine, so
frac is built from is_ge + scalar_tensor_tensor (all exact in fp32):
  f_0 = c * 0.5                      # c in [0,1) so already fractional
  f_i = 2*f_{i-1} - (f_{i-1} >= 0.5)
  g_i = (f_i + 0.25) - (f_i >= 0.75)
f and g are interleaved into a 6-wide tile so one ACT Sin per frequency
produces the contiguous [sin j0, cos j0, sin j1, cos j1, sin j2, cos j2]
output block at channel 3+6i.
"""
import math

import numpy as np

import concourse.bass as bass
import concourse.mybir as mybir
import concourse.tile as tile
from concourse import bass_utils
from concourse.alu_op_type import AluOpType

N_FULL = 2097152
N_CORES = 8
N_CORE = N_FULL // N_CORES      # 262144 points per core
P = 128                         # SBUF partitions
NPP = N_CORE // P               # 2048 points per partition
T = 256                         # points per partition per tile
NT = NPP // T                   # 8 tiles
NF = 10                         # frequencies
OUT_DIM = 63


class SplitWaitTileContext(tile.TileContext):
    """Walrus build accepts at most ONE sync-wait per instruction; split extras
    onto standalone single-wait EventSemaphore instructions."""

    def _sem_handles_by_num(self):
        assert self.sems is not None
        return {h.num: h for h in self.sems.allocated().values()}

    def _lower_ordered_insts(self, ordered):
        by_num = self._sem_handles_by_num()
        hook = self.nc._state.pop_inst_callback()
        assert hook == self._instruction_hook
        try:
            for insts in ordered.values():
                out = []
                for inst in insts:
                    si = inst.sync_info
                    if si is not None and si.on_wait and len(si.on_wait) > 1:
                        extra = list(si.on_wait[:-1])
                        del si.on_wait[:-1]
                        for w in extra:
                            assert w.wait_mode in ("sem-ge-imm", "sem-ge"), (
                                inst.name,
                                w.wait_mode,
                            )
                            ev = self.nc.engines[inst.engine].wait_ge(
                                by_num[w.id], w.wait_value
                            )
                            out.append(ev.ins)
                    out.append(inst)
                insts[:] = out
        finally:
            self.nc._state.push_inst_callback(hook)
        super()._lower_ordered_insts(ordered)

    def _drain_and_barrier(self, tick_clock, wait_clock):
        drain = self.nc.sync.drain()
        wait_clock.add_sem_waits(
            drain.ins, tile.ScopedClock({None: tick_clock.global_clock})
        )
        si = drain.ins.sync_info
        if si is not None and si.on_wait and len(si.on_wait) > 1:
            by_num = self._sem_handles_by_num()
            extra = list(si.on_wait[1:])
            del si.on_wait[1:]
            for w in extra:
                assert w.wait_mode in ("sem-ge-imm", "sem-ge"), w.wait_mode
                self.nc.sync.wait_ge(by_num[w.id], w.wait_value)
        self.nc.all_engine_barrier()
        assert self.sems is not None
        popped = self.nc._tile_sem_poison_stack.pop()
        assert popped is self._sem_poison
        self.nc.clear_and_free_semaphores(list(self.sems.allocated().values()))
        self.nc.all_engine_barrier()


def _repeat_tiles(repeat):
    for _ in range(repeat):
        yield from range(NT)


def build(repeat=1):
    nc = bass.Bass("TRN2")
    x = nc.dram_tensor("coords", (N_CORE, 3), mybir.dt.float32, kind="ExternalInput")
    y = nc.dram_tensor("out", (N_CORE, OUT_DIM), mybir.dt.float32, kind="ExternalOutput")
    xv = x.rearrange("(p n) d -> p n d", p=P)   # [128, 2048, 3]
    yv = y.rearrange("(p n) d -> p n d", p=P)   # [128, 2048, 63]
    pi_t = nc.alloc_sbuf_tensor("const-float32-pi", [P, 1], mybir.dt.float32)
    nc.gpsimd.memset(pi_t.ap(), math.pi)
    nc.const_aps.aps[(mybir.dt.float32, math.pi)] = pi_t.ap()
    nc.all_engine_barrier()
    with SplitWaitTileContext(nc) as tc:
        with tc.tile_pool(name="xin", bufs=1) as xpool, \
             tc.tile_pool(name="work", bufs=3) as wpool, \
             tc.tile_pool(name="bsel", bufs=2) as bpool, \
             tc.tile_pool(name="outp", bufs=2) as opool:
            xin = xpool.tile((P, NPP, 3), mybir.dt.float32, tag="xin")
            nc.sync.dma_start(xin, xv)
            for tt in _repeat_tiles(repeat):
                xt = xin[:, tt * T:(tt + 1) * T, :]
                ot = opool.tile((P, T, OUT_DIM), mybir.dt.float32, tag="ot")
                nc.gpsimd.tensor_copy(ot[:, :, 0:3], xt)
                prev_f = None
                for i in range(NF):
                    h = wpool.tile((P, T, 6), mybir.dt.float32, tag="h")
                    fs = h[:, :, 0:6:2]
                    gs = h[:, :, 1:6:2]
                    if i == 0:
                        nc.vector.tensor_scalar(
                            fs, xt, 0.5, None, AluOpType.mult
                        )
                    else:
                        b = bpool.tile((P, T, 3), mybir.dt.float32, tag="b")
                        nc.vector.tensor_scalar(
                            b, prev_f, 0.5, None, AluOpType.is_ge
                        )
                        nc.vector.scalar_tensor_tensor(
                            fs, prev_f, 2.0, b, AluOpType.mult, AluOpType.subtract
                        )
                    b2 = bpool.tile((P, T, 3), mybir.dt.float32, tag="b2")
                    nc.vector.tensor_scalar(
                        b2, fs, 0.75, None, AluOpType.is_ge
                    )
                    nc.vector.scalar_tensor_tensor(
                        gs, fs, 0.25, b2, AluOpType.add, AluOpType.subtract
                    )
                    nc.scalar.activation(
                        ot[:, :, 3 + 6 * i:9 + 6 * i],
                        h,
                        mybir.ActivationFunctionType.Sin,
                        bias=math.pi,
                        scale=-2.0 * math.pi,
                    )
                    prev_f = fs
                nc.sync.dma_start(yv[:, tt * T:(tt + 1) * T, :], ot)
    return nc


def kernel(coords: np.ndarray) -> np.ndarray:
    full = np.ascontiguousarray(coords.reshape(N_FULL, 3), dtype=np.float32)
    ins = [
        {"coords": full[c * N_CORE:(c + 1) * N_CORE]} for c in range(N_CORES)
    ]
    nc = build()
    res = bass_utils.run_bass_kernel_spmd(nc, ins, core_ids=list(range(N_CORES)))
    outs = [np.asarray(res.results[c]["out"]) for c in range(N_CORES)]
    return np.concatenate(outs, axis=0).reshape(1, N_FULL, OUT_DIM)


# revision 21
# speedup vs baseline: 1.2204x; 1.2204x over previous
"""PosEncoding TRN2 kernel: out = [c, sin(2^i pi c_j), cos(2^i pi c_j)] interleaved.

Full input coords [1, 2097152, 3] f32 -> output [1, 2097152, 63] f32.
8-way data parallel over the points axis (262144 points/core).

Math per element (freq i in 0..9, feature j in 0..2):
  f_i = frac(c * 2^(i-1))            # exact fp32 chain, see below
  sin(2^i pi c) = Sin(-2pi f_i + pi) # arg in (-pi, pi], ACT valid range
  g_i = frac(f_i + 0.25)
  cos(2^i pi c) = Sin(-2pi g_i + pi)
This walrus build's ISA check rejects AluOpType.mod on every engine, so
frac is built from is_ge + scalar_tensor_tensor (all exact in fp32):
  f_0 = c * 0.5                      # c in [0,1) so already fractional
  f_i = 2*f_{i-1} - (f_{i-1} >= 0.5)
  g_i = (f_i + 0.25) - (f_i >= 0.75)
f and g are interleaved into a 6-wide tile so one ACT Sin per frequency
produces the contiguous [sin j0, cos j0, sin j1, cos j1, sin j2, cos j2]
output block at channel 3+6i.

Engine balance (default mode full+gact5): the all-DVE g-chain makes DVE
the critical path (~39 ops/tile > DMA floor), so for freqs >= 5 the cos
side moves to ACT via cos(2pi f) = Sin(2pi|f - 0.5| - pi/2) with
|f - 0.5| = ACT Abs(bias=-0.5); sin comes straight off f. That splits
work ~evenly (DVE ~29 ops/tile, ACT ~25) and benches at the DMA
roofline (~196us/rep = 66MB out @ ~400GB/s/core; dma-only ablation is
identical within noise). gact0 (all-ACT cos) is ACT-bound and slower.
"""
import math

import numpy as np

import concourse.bass as bass
import concourse.mybir as mybir
import concourse.tile as tile
from concourse import bass_utils
from concourse.alu_op_type import AluOpType

N_FULL = 2097152
N_CORES = 8
N_CORE = N_FULL // N_CORES      # 262144 points per core
P = 128                         # SBUF partitions
NPP = N_CORE // P               # 2048 points per partition
T = 256                         # points per partition per tile
NT = NPP // T                   # 8 tiles
NF = 10                         # frequencies
OUT_DIM = 63


class SplitWaitTileContext(tile.TileContext):
    """Walrus build accepts at most ONE sync-wait per instruction; split extras
    onto standalone single-wait EventSemaphore instructions."""

    def _sem_handles_by_num(self):
        assert self.sems is not None
        return {h.num: h for h in self.sems.allocated().values()}

    def _lower_ordered_insts(self, ordered):
        by_num = self._sem_handles_by_num()
        hook = self.nc._state.pop_inst_callback()
        assert hook == self._instruction_hook
        try:
            for insts in ordered.values():
                out = []
                for inst in insts:
                    si = inst.sync_info
                    if si is not None and si.on_wait and len(si.on_wait) > 1:
                        extra = list(si.on_wait[:-1])
                        del si.on_wait[:-1]
                        for w in extra:
                            assert w.wait_mode in ("sem-ge-imm", "sem-ge"), (
                                inst.name,
                                w.wait_mode,
                            )
                            ev = self.nc.engines[inst.engine].wait_ge(
                                by_num[w.id], w.wait_value
                            )
                            out.append(ev.ins)
                    out.append(inst)
                insts[:] = out
        finally:
            self.nc._state.push_inst_callback(hook)
        super()._lower_ordered_insts(ordered)

    def _drain_and_barrier(self, tick_clock, wait_clock):
        drain = self.nc.sync.drain()
        wait_clock.add_sem_waits(
            drain.ins, tile.ScopedClock({None: tick_clock.global_clock})
        )
        si = drain.ins.sync_info
        if si is not None and si.on_wait and len(si.on_wait) > 1:
            by_num = self._sem_handles_by_num()
            extra = list(si.on_wait[1:])
            del si.on_wait[1:]
            for w in extra:
                assert w.wait_mode in ("sem-ge-imm", "sem-ge"), w.wait_mode
                self.nc.sync.wait_ge(by_num[w.id], w.wait_value)
        self.nc.all_engine_barrier()
        assert self.sems is not None
        popped = self.nc._tile_sem_poison_stack.pop()
        assert popped is self._sem_poison
        self.nc.clear_and_free_semaphores(list(self.sems.allocated().values()))
        self.nc.all_engine_barrier()


def _repeat_tiles(repeat):
    for _ in range(repeat):
        yield from range(NT)


def build(repeat=1, mode="full+gact5", bench_sink=False):
    parts = mode.split("+")
    base = parts[0]
    flags = set(parts[1:])
    gk = NF  # freqs < gk use DVE g-chain; >= gk use ACT Abs cos path
    for fl in flags:
        if fl.startswith("gact"):
            gk = int(fl[4:])
    nc = bass.Bass("TRN2")
    x = nc.dram_tensor("coords", (N_CORE, 3), mybir.dt.float32, kind="ExternalInput")
    y = nc.dram_tensor(
        "out", (N_CORE, OUT_DIM), mybir.dt.float32,
        kind="Internal" if bench_sink else "ExternalOutput",
    )
    if bench_sink:
        small = nc.dram_tensor(
            "out_small", (P, 1, 3), mybir.dt.float32, kind="ExternalOutput"
        )
    xv = x.rearrange("(p n) d -> p n d", p=P)   # [128, 2048, 3]
    yv = y.rearrange("(p n) d -> p n d", p=P)   # [128, 2048, 63]
    for cname, cval in (
        ("pi", math.pi),
        ("nhalfpi", -math.pi / 2.0),
        ("nhalf", -0.5),
    ):
        ct = nc.alloc_sbuf_tensor(f"const-float32-{cname}", [P, 1], mybir.dt.float32)
        nc.gpsimd.memset(ct.ap(), cval)
        nc.const_aps.aps[(mybir.dt.float32, cval)] = ct.ap()
    nc.all_engine_barrier()
    with SplitWaitTileContext(nc) as tc:
        with tc.tile_pool(name="xin", bufs=1) as xpool, \
             tc.tile_pool(name="work", bufs=3) as wpool, \
             tc.tile_pool(name="bsel", bufs=2) as bpool, \
             tc.tile_pool(name="outp", bufs=2) as opool:
            xin = xpool.tile((P, NPP, 3), mybir.dt.float32, tag="xin")
            nc.sync.dma_start(xin, xv)
            if bench_sink:
                nc.sync.dma_start(small[:, :, :], xin[:, 0:1, :])
            for tt in _repeat_tiles(repeat):
                xt = xin[:, tt * T:(tt + 1) * T, :]
                ot = opool.tile((P, T, OUT_DIM), mybir.dt.float32, tag="ot")
                if "actcopy" in flags:
                    nc.scalar.copy(ot[:, :, 0:3], xt)
                elif "dvecopy" in flags:
                    nc.vector.tensor_copy(ot[:, :, 0:3], xt)
                else:
                    nc.gpsimd.tensor_copy(ot[:, :, 0:3], xt)
                beng = nc.gpsimd if "poolb" in flags else nc.vector
                if base == "nodve":
                    h0 = wpool.tile((P, T, 6), mybir.dt.float32, tag="h")
                    nc.vector.tensor_scalar(
                        h0[:, :, 0:6:2], xt, 0.5, None, AluOpType.mult
                    )
                    nc.vector.tensor_scalar(
                        h0[:, :, 1:6:2], xt, 0.25, None, AluOpType.mult
                    )
                prev_f = None
                for i in range(NF):
                    if base == "dma":
                        break
                    if base == "nodve":
                        nc.scalar.activation(
                            ot[:, :, 3 + 6 * i:9 + 6 * i],
                            h0,
                            mybir.ActivationFunctionType.Sin,
                            bias=math.pi,
                            scale=-2.0 * math.pi,
                        )
                        continue
                    if i >= gk:
                        ft = wpool.tile((P, T, 3), mybir.dt.float32, tag="f")
                        if i == 0:
                            nc.vector.tensor_scalar(
                                ft, xt, 0.5, None, AluOpType.mult
                            )
                        else:
                            b = bpool.tile((P, T, 3), mybir.dt.float32, tag="b")
                            beng.tensor_scalar(
                                b, prev_f, 0.5, None, AluOpType.is_ge
                            )
                            nc.vector.scalar_tensor_tensor(
                                ft, prev_f, 2.0, b,
                                AluOpType.mult, AluOpType.subtract,
                            )
                        if base != "noact":
                            w = bpool.tile((P, T, 3), mybir.dt.float32, tag="w")
                            nc.scalar.activation(
                                w, ft, mybir.ActivationFunctionType.Abs,
                                bias=-0.5, scale=1.0,
                            )
                            nc.scalar.activation(
                                ot[:, :, 3 + 6 * i:9 + 6 * i:2], ft,
                                mybir.ActivationFunctionType.Sin,
                                bias=math.pi, scale=-2.0 * math.pi,
                            )
                            nc.scalar.activation(
                                ot[:, :, 4 + 6 * i:9 + 6 * i:2], w,
                                mybir.ActivationFunctionType.Sin,
                                bias=-math.pi / 2.0, scale=2.0 * math.pi,
                            )
                        prev_f = ft
                        continue
                    h = wpool.tile((P, T, 6), mybir.dt.float32, tag="h")
                    fs = h[:, :, 0:6:2]
                    gs = h[:, :, 1:6:2]
                    if i == 0:
                        nc.vector.tensor_scalar(
                            fs, xt, 0.5, None, AluOpType.mult
                        )
                    else:
                        b = bpool.tile((P, T, 3), mybir.dt.float32, tag="b")
                        beng.tensor_scalar(
                            b, prev_f, 0.5, None, AluOpType.is_ge
                        )
                        nc.vector.scalar_tensor_tensor(
                            fs, prev_f, 2.0, b, AluOpType.mult, AluOpType.subtract
                        )
                    b2 = bpool.tile((P, T, 3), mybir.dt.float32, tag="b2")
                    beng.tensor_scalar(
                        b2, fs, 0.75, None, AluOpType.is_ge
                    )
                    nc.vector.scalar_tensor_tensor(
                        gs, fs, 0.25, b2, AluOpType.add, AluOpType.subtract
                    )
                    if base != "noact":
                        nc.scalar.activation(
                            ot[:, :, 3 + 6 * i:9 + 6 * i],
                            h,
                            mybir.ActivationFunctionType.Sin,
                            bias=math.pi,
                            scale=-2.0 * math.pi,
                        )
                    prev_f = fs
                deng = nc.scalar if ("dmasplit" in flags and tt % 2) else nc.sync
                deng.dma_start(yv[:, tt * T:(tt + 1) * T, :], ot)
    return nc


def kernel(coords: np.ndarray) -> np.ndarray:
    full = np.ascontiguousarray(coords.reshape(N_FULL, 3), dtype=np.float32)
    ins = [
        {"coords": full[c * N_CORE:(c + 1) * N_CORE]} for c in range(N_CORES)
    ]
    nc = build()
    res = bass_utils.run_bass_kernel_spmd(nc, ins, core_ids=list(range(N_CORES)))
    outs = [np.asarray(res.results[c]["out"]) for c in range(N_CORES)]
    return np.concatenate(outs, axis=0).reshape(1, N_FULL, OUT_DIM)
